# revision 45
# baseline (speedup 1.0000x reference)
"""Trainium2 Bass kernel: GPT2-style windowed attention (DecisionTransformer).

Full-input contract: kernel(**inputs) -> [B, S, D] float32.

Sharding: batch*heads across 8 cores (core c -> batch c//4, heads 4*(c%4)..+4).
Each core: column-sliced c_attn, full windowed attention for its 4 heads,
row-sliced c_proj producing a partial [S, D] output; host sums partials
(the "all-reduce") and adds c_proj bias + V-bias contribution once.

Layout / schedule choices:
  - all matmul operands are bf16 (tolerance is 2e-2; bf16 keeps PE at
    1 cyc/row and halves HBM traffic). PSUM accumulation stays fp32.
  - hidden is sent pre-transposed (xT [D, S]) so QK projections emit
    qT/kT directly in [head*dim, seq] layout; V is projected in
    [seq, head*dim] layout with a ones-column per head so attn@V
    accumulates softmax denominators in PSUM row 64 for free. V-bias
    folds into the host-side reduce (softmax rows sum to 1).
  - the whole kernel is software-pipelined: projections advance per
    512-col seq chunk, and attention q-quarters (scores+exp for all 4
    heads, kj-major attn@V, normalize, c_proj slice, output DMA) are
    issued as soon as their chunk dependencies are met. ACT does only
    exp during attention (the gating engine); evacuations go to DVE.
  - rope: rotate_half via 4 cross-quadrant 32-partition copies (2 DVE,
    2 GPSIMD) -- no DMA, no partition-swap latency chain.
  - scores for a head pair (partitions 0-63 / 64-127, K=64 each) are
    issued back-to-back so the PE runs them as concurrent row-tiles.
  - a dozen dummy matmuls at t=0 keep the PE busy (and the HAM
    clock-gate open) while the first input DMAs land.
"""

import sys

import numpy as np

sys.path.insert(0, "/opt/trn_rl_repo")

B, S, D = 2, 2048, 1024
H, HD = 16, 64
WINDOW = 512
ROPE_BASE = 4000.0
NCORES = 8
NH = 4          # heads per core
KT = D // 128   # 8 contraction tiles for c_attn
NB = S // 128   # 16 seq blocks
WB = WINDOW // 128  # 4 -> band spans up to 5 query blocks per key block


def _build_nc(debug_taps=False):
    import concourse.bass as bass
    from concourse import bacc, library_config, mybir
    import concourse.tile as tile

    f32 = mybir.dt.float32
    bf16 = mybir.dt.bfloat16
    Exp = mybir.ActivationFunctionType.Exp
    mult = mybir.AluOpType.mult
    ts = bass.ts
    ds = bass.ds

    nc = bacc.Bacc("TRN2")

    xT_d = nc.dram_tensor("xT", [D, S], bf16, kind="ExternalInput")
    wqkv_d = nc.dram_tensor("wqkv", [D, 3 * NH * HD], bf16, kind="ExternalInput")
    bqk_d = nc.dram_tensor("bqk", [128, 4], f32, kind="ExternalInput")
    wp_d = nc.dram_tensor("wp", [NH * HD, D], bf16, kind="ExternalInput")
    cos2_d = nc.dram_tensor("cos2", [128, S], bf16, kind="ExternalInput")
    sin2_d = nc.dram_tensor("sin2", [128, S], bf16, kind="ExternalInput")
    m04_d = nc.dram_tensor("m04", [128, 256], bf16, kind="ExternalInput")
    out_d = nc.dram_tensor("out", [S, D], bf16, kind="ExternalOutput")

    HS = S // 4  # q-quarter span: po is 1 PSUM bank
    QB = NB // 4  # 4 q-blocks per quarter

    with tile.TileContext(nc) as tc:
        nc.gpsimd.load_library(library_config.attn)

        with (
            tc.tile_pool(name="persist", bufs=1) as pp,
            tc.tile_pool(name="psj", bufs=2, space="PSUM") as psj_pool,
            tc.tile_pool(name="ps", bufs=2, space="PSUM") as ps_pool,
            tc.tile_pool(name="pso", bufs=2, space="PSUM") as pso_pool,
            tc.tile_pool(name="xw", bufs=1) as xw_pool,
            tc.tile_pool(name="ropetmp", bufs=4) as tmp_pool,
            tc.tile_pool(name="et", bufs=36) as e_pool,
            tc.tile_pool(name="rb", bufs=3) as rb_pool,
            tc.tile_pool(name="yo", bufs=3) as y_pool,
        ):
            # prewarm tile: zeros, matmul'd while input DMAs land
            zb = pp.tile([128, 512], bf16, tag="zb")
            nc.vector.memset(zb[:], 0.0)

            bqk_t = pp.tile([128, 4], f32, tag="bqk")
            nc.sync.dma_start(bqk_t[:], bqk_d[:])
            # m04[:, 0, :] = diag-block mask (kk<=qq); [:, 1, :] = window
            # edge mask (kk>qq) — applied as one two-region strided op
            m04t = pp.tile([128, 2, 128], bf16, tag="m04")
            nc.sync.dma_start(
                m04t[:], m04_d[:].rearrange("p (a b) -> p a b", a=2)
            )

            qk = [
                pp.tile([128, S], bf16, tag=f"qk{c}", name=f"qk{c}") for c in range(4)
            ]
            # per-head K tiles, zero-padded to K=128 (rows 64-127 = 0) so
            # score matmuls are full-partition and pipeline at N-cycle
            # rate; the other head's q rows are nulled by the zero k rows.
            kp = [
                pp.tile([128, S], bf16, tag=f"kp{h}", name=f"kp{h}")
                for h in range(NH)
            ]
            # the k data sits in the same partition half as the head's q
            # rows (even: 0-63, odd: 64-127); the other half is zero
            for h in range(NH):
                hb = (h % 2) * 64
                nc.vector.memset(kp[h][64 - hb : 128 - hb, :], 0.0)
            CV = NH * 65  # 260: per head 64 v-cols + 1 ones col
            vbig = pp.tile([128, NB, CV], bf16, tag="vbig")
            outH = pp.tile([128, 2, S], bf16, tag="outH")
            wpt = pp.tile([128, 2, D], bf16, tag="wpt")

            wbig = xw_pool.tile([128, KT, 3 * NH * HD], bf16, tag="wbig")
            xbig = xw_pool.tile([128, KT, S], bf16, tag="xbig")
            cos2 = xw_pool.tile([128, S], bf16, tag="cos2")
            sin2 = xw_pool.tile([128, S], bf16, tag="sin2")

            # DMA order = need order: v-cols, x first chunk, qk-cols, rope
            # tables, rest of x, c_proj weights.
            # interleave w rows with the first x slice per kt so the first
            # V-projection matmul group can start as soon as kt 0 lands
            for kt in range(KT):
                nc.sync.dma_start(wbig[:, kt, :], wqkv_d[ts(kt, 128), :])
                nc.sync.dma_start(xbig[:, kt, 0:512], xT_d[ts(kt, 128), 0:512])
            nc.sync.dma_start(cos2[:], cos2_d[:])
            nc.sync.dma_start(sin2[:], sin2_d[:])
            for kt in range(KT):
                nc.sync.dma_start(xbig[:, kt, 512:1024], xT_d[ts(kt, 128), 512:1024])
            for kt in range(KT):
                nc.sync.dma_start(
                    xbig[:, kt, 1024:2048], xT_d[ts(kt, 128), 1024:2048]
                )
            for k2 in range(2):
                nc.sync.dma_start(wpt[:, k2, :], wp_d[ts(k2, 128), :])

            # PE prewarm: keep the tensor engine busy (and the HAM
            # clock-gate open) while the first input DMAs land.
            for w in range(16):
                psw = psj_pool.tile([128, 512], f32, tag="psj", name="psw")
                nc.tensor.matmul(
                    psw[:], zb[:, 0:128], zb[:, 0:512],
                    start=True, stop=True,
                )

            # ---------------- building blocks ----------------
            eTs = [dict() for _ in range(NH)]  # [h][kj] -> masked exp'd scores

            def proj_v(sb):
                vsb = vbig[:, sb, :].rearrange("p (h c) -> p h c", c=65)
                nc.vector.memset(vsb[:, :, 64:65], 1.0)
                psv = psj_pool.tile([128, 256], f32, tag="psj", name="psv")
                for kt in range(KT):
                    nc.tensor.matmul(
                        psv[:],
                        xbig[:, kt, ts(sb, 128)],
                        wbig[:, kt, ds(2 * NH * HD, NH * HD)],
                        start=(kt == 0),
                        stop=(kt == KT - 1),
                    )
                # alternate evac engine so neither ACT nor DVE builds a
                # backlog in front of the psum pool recycle
                if sb % 2 == 0:
                    nc.vector.tensor_copy(
                        vsb[:, :, 0:64],
                        psv[:].rearrange("p (h c) -> p h c", c=64),
                    )
                else:
                    nc.scalar.copy(
                        vsb[:, :, 0:64],
                        psv[:].rearrange("p (h c) -> p h c", c=64),
                    )

            def proj_qk(c, sc):
                psb = psj_pool.tile([128, 512], f32, tag="psj", name="psb")
                for kt in range(KT):
                    nc.tensor.matmul(
                        psb[:],
                        wbig[:, kt, ts(c, 128)],
                        xbig[:, kt, ts(sc, 512)],
                        start=(kt == 0),
                        stop=(kt == KT - 1),
                    )
                nc.scalar.add(
                    qk[c][:, ts(sc, 512)], psb[:], bqk_t[:, c : c + 1]
                )
                # rope: rotate_half via partition-swap SBUF DMAs on the
                # sync/gpsimd queues (sign is folded into the sin table)
                qc = qk[c][:, ts(sc, 512)]
                tmp = tmp_pool.tile([128, 512], bf16, tag="ropetmp", name="tmp")
                dma_engs = [nc.sync, nc.gpsimd, nc.sync, nc.gpsimd]
                for g in range(2):
                    b0 = g * 64
                    dma_engs[2 * g].dma_start(
                        tmp[b0 : b0 + 32, :],
                        qk[c][b0 + 32 : b0 + 64, ts(sc, 512)],
                    )
                    dma_engs[2 * g + 1].dma_start(
                        tmp[b0 + 32 : b0 + 64, :],
                        qk[c][b0 : b0 + 32, ts(sc, 512)],
                    )
                nc.vector.tensor_tensor(
                    tmp[:], tmp[:], sin2[:, ts(sc, 512)], op=mult
                )
                nc.vector.tensor_tensor(qc, qc, cos2[:, ts(sc, 512)], op=mult)
                nc.vector.tensor_add(qc, qc, tmp[:])
                if c >= 2:
                    # spread each roped k head into its padded K=128 tile
                    # (partition-aligned with the head's q rows)
                    h0 = 2 * (c - 2)
                    nc.sync.dma_start(
                        kp[h0][0:64, ts(sc, 512)], qk[c][0:64, ts(sc, 512)]
                    )
                    nc.gpsimd.dma_start(
                        kp[h0 + 1][64:128, ts(sc, 512)], qk[c][64:128, ts(sc, 512)]
                    )

            def scores_mm(h, kj, pss, nq, part):
                # one scores matmul sT[k, q]; part 0 = first 512 q-cols,
                # part 1 = the 128-col band tail. lhsT is the head's
                # zero-padded K=128 tile; the moving q carries both heads'
                # rows (the foreign head is nulled by the zero k rows).
                qt = qk[h // 2]
                n1 = min(512, nq * 128)
                n2 = nq * 128 - n1
                lhs_k = kp[h][:, ts(kj, 128)]
                if part == 0:
                    nc.tensor.matmul(
                        pss[:, 0:n1],
                        lhs_k,
                        qt[:, ds(kj * 128, n1)],
                        start=True,
                        stop=True,
                    )
                elif n2:
                    nc.tensor.matmul(
                        pss[:, 512 : 512 + n2],
                        lhs_k,
                        qt[:, ds(kj * 128 + 512, n2)],
                        start=True,
                        stop=True,
                    )

            def exp_mask(h, kj, pss, nq):
                eT = e_pool.tile([128, 640], bf16, tag="et", name="eT")
                nc.scalar.activation(
                    eT[:, 0 : nq * 128], pss[:, 0 : nq * 128], Exp, scale=0.125
                )
                # banded mask: diag block keeps kk<=qq, window edge kk>qq.
                # Both 128-col regions are masked in one strided DVE op.
                if nq == WB + 1:
                    e2 = eT[:, 0:640].rearrange("p (x y) -> p x y", y=128)[
                        :, 0:5:4, :
                    ]
                    nc.vector.tensor_tensor(e2, e2, m04t[:], op=mult)
                else:
                    nc.vector.tensor_tensor(
                        eT[:, 0:128], eT[:, 0:128], m04t[:, 0, :], op=mult
                    )
                eTs[h][kj] = eT

            def evac_q(po, h, qtr):
                # normalize by denominators (PSUM row 64) into outH.
                # The denom row is staged via ACT while ACT has slack
                # (early waves); via DVE once ACT is exp-bound late.
                hb = (h % 2) * 64
                rb = rb_pool.tile([64, HS], f32, tag="rb", name="rb")
                if qtr == 2:
                    nc.vector.tensor_copy(rb[0:1, :], po[64:65, :])
                else:
                    nc.scalar.copy(rb[0:1, :], po[64:65, :])
                nc.vector.reciprocal_approx_fast(rb[0:1, :], rb[0:1, :])
                nc.gpsimd.partition_broadcast(rb[:], rb[0:1, :])
                nc.vector.tensor_tensor(
                    outH[hb : hb + 64, h // 2, qtr * HS : (qtr + 1) * HS],
                    po[0:64, :],
                    rb[:],
                    op=mult,
                )

            def attnv(h, qtr):
                # kj-major attn@V: each V block loads once and streams its
                # whole q-span (per-element has_written bits handle the
                # staggered accumulation regions)
                kjlo = max(0, 4 * qtr - WB)
                kjhi = 4 * qtr + QB - 1
                po = pso_pool.tile([65, HS], f32, tag="pso", name="po")
                for kj in range(kjlo, kjhi + 1):
                    qlo = max(4 * qtr, kj)
                    qhi = min(4 * qtr + QB - 1, kj + WB)
                    off = (qlo - kj) * 128
                    n = (qhi - qlo + 1) * 128
                    nc.tensor.matmul(
                        po[:, ds((qlo - 4 * qtr) * 128, n)],
                        vbig[:, kj, h * 65 : h * 65 + 65],
                        eTs[h][kj][:, off : off + n],
                        start=(kj == kjlo),
                        stop=(kj == kjhi),
                    )
                evac_q(po, h, qtr)

            def cproj(sb):
                # two 1-bank psum groups so c_proj never contends with the
                # scores pool
                yt = y_pool.tile([128, D], bf16, tag="yo", name="yt")
                for ncol in range(2):
                    psp = psj_pool.tile([128, 512], f32, tag="psj", name="psp")
                    for k2 in range(2):
                        nc.tensor.matmul(
                            psp[:],
                            outH[:, k2, ts(sb, 128)],
                            wpt[:, k2, ts(ncol, 512)],
                            start=(k2 == 0),
                            stop=(k2 == 1),
                        )
                    if (sb + ncol) % 2 == 0:
                        nc.scalar.copy(yt[:, ts(ncol, 512)], psp[:])
                    else:
                        nc.vector.tensor_copy(yt[:, ts(ncol, 512)], psp[:])
                nc.sync.dma_start(out_d[ts(sb, 128), :], yt[:])

            def scores_kj(kj):
                # scores+exp for one key block, head-pair interleaved
                nq = min(WB + 1, NB - kj)
                for pair in range(2):
                    h0, h1 = 2 * pair, 2 * pair + 1
                    p0 = ps_pool.tile([128, 640], f32, tag="ps", name="pss")
                    scores_mm(h0, kj, p0, nq, 0)
                    scores_mm(h0, kj, p0, nq, 1)
                    p1 = ps_pool.tile([128, 640], f32, tag="ps", name="pss")
                    scores_mm(h1, kj, p1, nq, 0)
                    scores_mm(h1, kj, p1, nq, 1)
                    exp_mask(h0, kj, p0, nq)
                    exp_mask(h1, kj, p1, nq)

            def wave(qtr):
                # Scores run a full wave ahead: this wave consumes quarter
                # qtr's eT tiles (exp'd during wave qtr-1) and produces
                # quarter qtr+1's. Projection chunks lead the wave so the
                # PE stays dense (and the clock-gate warm) while ACT drains
                # the exp backlog; attn@V heads, new score blocks, and the
                # previous quarter's c_proj interleave so no engine queue
                # builds a deep backlog in front of a dependency.
                ch = qtr + 2
                if ch < 4:
                    for j in range(4):
                        proj_v(4 * ch + j)
                        proj_qk((0, 2, 1, 3)[j], ch)
                # c_proj of the previous quarter starts one slot late so
                # its first group never waits on that quarter's last
                # normalization chain
                for i in range(4):
                    if qtr >= 1 and i >= 1:
                        cproj(4 * (qtr - 1) + i - 1)
                    attnv(i, qtr)
                    if qtr < 3:
                        scores_kj(4 * qtr + 4 + i)
                if qtr >= 1:
                    cproj(4 * (qtr - 1) + 3)

            # ---------------- pipelined schedule ----------------
            for sc in range(2):
                for sb in range(4 * sc, 4 * sc + 4):
                    proj_v(sb)
                for c in (0, 2, 1, 3):
                    proj_qk(c, sc)
            for kj in range(4):
                scores_kj(kj)
            for qtr in range(4):
                wave(qtr)
            for sb in range(12, 16):
                cproj(sb)

    nc.compile()
    return nc


def _host_inputs(hidden, pos, caw, cab, cpw):
    """Build the 8 per-core input maps."""
    inv = 1.0 / (ROPE_BASE ** (np.arange(0, HD, 2, dtype=np.float32) / HD))
    t = np.arange(S, dtype=np.float32)
    freqs = np.outer(t, inv).astype(np.float32)
    emb = np.concatenate([freqs, freqs], axis=1)  # [S, HD]
    cos = np.cos(emb).astype(np.float32)
    sin = np.sin(emb).astype(np.float32)

    import ml_dtypes

    bf = ml_dtypes.bfloat16
    ii = np.arange(128)
    m0 = (ii[:, None] <= ii[None, :]).astype(bf)
    m4 = (ii[:, None] > ii[None, :]).astype(bf)
    m04 = np.ascontiguousarray(np.concatenate([m0, m4], axis=1))

    xTs, cos2s, sin2s = [], [], []
    for b in range(B):
        xTs.append(np.ascontiguousarray(hidden[b].T).astype(bf))
        cosT = np.ascontiguousarray(cos[pos[b]].T)  # [HD, S]
        sinT = np.ascontiguousarray(sin[pos[b]].T)
        sinS = np.concatenate([-sinT[:32], sinT[32:]], axis=0)
        cos2s.append(np.tile(cosT, (2, 1)).astype(bf))
        sin2s.append(np.tile(sinS, (2, 1)).astype(bf))

    in_maps = []
    for c in range(NCORES):
        b = c // 4
        h0 = NH * (c % 4)
        col = h0 * HD
        w_q = caw[:, col : col + NH * HD]
        w_k = caw[:, D + col : D + col + NH * HD]
        w_v = caw[:, 2 * D + col : 2 * D + col + NH * HD]
        wqkv = np.ascontiguousarray(
            np.concatenate([w_q, w_k, w_v], axis=1)
        ).astype(bf)
        b_q = cab[col : col + NH * HD]
        b_k = cab[D + col : D + col + NH * HD]
        bqk = np.ascontiguousarray(
            np.concatenate([b_q, b_k]).reshape(4, 128).T
        )  # [128, 4]: partition = col within tile
        wp = np.ascontiguousarray(cpw[col : col + NH * HD, :]).astype(bf)
        in_maps.append(
            {
                "xT": xTs[b],
                "wqkv": wqkv,
                "bqk": bqk,
                "wp": wp,
                "cos2": cos2s[b],
                "sin2": sin2s[b],
                "m04": m04,
            }
        )
    return in_maps


def _assemble(results, cab, cpw, cpb):
    """Host all-reduce of the 4 per-batch partials + biases.

    The V-bias contribution is position-independent after softmax
    (attn rows sum to 1), so it folds into a constant row vector:
    bias_v @ c_proj_w.
    """
    vrow = cab[2 * D :].astype(np.float32) @ cpw.astype(np.float32)
    bias = cpb.astype(np.float32) + vrow
    y = np.empty((B, S, D), dtype=np.float32)
    for b in range(B):
        acc = results[4 * b]["out"].astype(np.float32)
        for c in range(4 * b + 1, 4 * b + 4):
            acc = acc + results[c]["out"].astype(np.float32)
        y[b] = acc + bias[None, :]
    return y


def kernel(**inputs):
    from concourse import bass_utils

    hidden = np.asarray(inputs["hidden_states"], dtype=np.float32)
    pos = np.asarray(inputs["position_ids"]).astype(np.int64)
    caw = np.asarray(inputs["c_attn_w"], dtype=np.float32)
    cab = np.asarray(inputs["c_attn_b"], dtype=np.float32)
    cpw = np.asarray(inputs["c_proj_w"], dtype=np.float32)
    cpb = np.asarray(inputs["c_proj_b"], dtype=np.float32)

    in_maps = _host_inputs(hidden, pos, caw, cab, cpw)
    nc = _build_nc()
    res = bass_utils.run_bass_kernel_spmd(nc, in_maps, list(range(NCORES)))
    return _assemble(res.results, cab, cpw, cpb)


# revision 47
# speedup vs baseline: 1.0121x; 1.0121x over previous
"""Trainium2 Bass kernel: GPT2-style windowed attention (DecisionTransformer).

Full-input contract: kernel(**inputs) -> [B, S, D] float32.

Sharding: batch*heads across 8 cores (core c -> batch c//4, heads 4*(c%4)..+4).
Each core: column-sliced c_attn, full windowed attention for its 4 heads,
row-sliced c_proj producing a partial [S, D] output; host sums partials
(the "all-reduce") and adds c_proj bias + V-bias contribution once.

Layout / schedule choices:
  - all matmul operands are bf16 (tolerance is 2e-2; bf16 keeps PE at
    1 cyc/row and halves HBM traffic). PSUM accumulation stays fp32.
  - hidden is sent pre-transposed (xT [D, S]) so QK projections emit
    qT/kT directly in [head*dim, seq] layout; V is projected in
    [seq, head*dim] layout with a ones-column per head so attn@V
    accumulates softmax denominators in PSUM row 64 for free. V-bias
    folds into the host-side reduce (softmax rows sum to 1).
  - the whole kernel is software-pipelined: projections advance per
    512-col seq chunk, and attention q-quarters (scores+exp for all 4
    heads, kj-major attn@V, normalize, c_proj slice, output DMA) are
    issued as soon as their chunk dependencies are met. ACT does only
    exp during attention (the gating engine); evacuations go to DVE.
  - rope: rotate_half via 4 cross-quadrant 32-partition copies (2 DVE,
    2 GPSIMD) -- no DMA, no partition-swap latency chain.
  - scores for a head pair (partitions 0-63 / 64-127, K=64 each) are
    issued back-to-back so the PE runs them as concurrent row-tiles.
  - a dozen dummy matmuls at t=0 keep the PE busy (and the HAM
    clock-gate open) while the first input DMAs land.
"""

import sys

import numpy as np

sys.path.insert(0, "/opt/trn_rl_repo")

B, S, D = 2, 2048, 1024
H, HD = 16, 64
WINDOW = 512
ROPE_BASE = 4000.0
NCORES = 8
NH = 4          # heads per core
KT = D // 128   # 8 contraction tiles for c_attn
NB = S // 128   # 16 seq blocks
WB = WINDOW // 128  # 4 -> band spans up to 5 query blocks per key block


def _build_nc(debug_taps=False):
    import concourse.bass as bass
    from concourse import bacc, library_config, mybir
    import concourse.tile as tile

    f32 = mybir.dt.float32
    bf16 = mybir.dt.bfloat16
    Exp = mybir.ActivationFunctionType.Exp
    mult = mybir.AluOpType.mult
    ts = bass.ts
    ds = bass.ds

    nc = bacc.Bacc("TRN2")

    xT_d = nc.dram_tensor("xT", [D, S], bf16, kind="ExternalInput")
    wqkv_d = nc.dram_tensor("wqkv", [D, 3 * NH * HD], bf16, kind="ExternalInput")
    bqk_d = nc.dram_tensor("bqk", [128, 4], f32, kind="ExternalInput")
    wp_d = nc.dram_tensor("wp", [NH * HD, D], bf16, kind="ExternalInput")
    cos2_d = nc.dram_tensor("cos2", [128, S], bf16, kind="ExternalInput")
    sin2_d = nc.dram_tensor("sin2", [128, S], bf16, kind="ExternalInput")
    m04_d = nc.dram_tensor("m04", [128, 256], bf16, kind="ExternalInput")
    out_d = nc.dram_tensor("out", [S, D], bf16, kind="ExternalOutput")

    HS = S // 4  # q-quarter span: po is 1 PSUM bank
    QB = NB // 4  # 4 q-blocks per quarter

    with tile.TileContext(nc) as tc:
        nc.gpsimd.load_library(library_config.attn)

        with (
            tc.tile_pool(name="persist", bufs=1) as pp,
            tc.tile_pool(name="psj", bufs=2, space="PSUM") as psj_pool,
            tc.tile_pool(name="ps", bufs=2, space="PSUM") as ps_pool,
            tc.tile_pool(name="pso", bufs=2, space="PSUM") as pso_pool,
            tc.tile_pool(name="xw", bufs=1) as xw_pool,
            tc.tile_pool(name="ropetmp", bufs=4) as tmp_pool,
            tc.tile_pool(name="et", bufs=36) as e_pool,
            tc.tile_pool(name="rb", bufs=3) as rb_pool,
            tc.tile_pool(name="yo", bufs=3) as y_pool,
        ):
            # prewarm tile: zeros, matmul'd while input DMAs land
            zb = pp.tile([128, 512], bf16, tag="zb")
            nc.vector.memset(zb[:], 0.0)

            bqk_t = pp.tile([128, 4], f32, tag="bqk")
            nc.sync.dma_start(bqk_t[:], bqk_d[:])
            # m04[:, 0, :] = diag-block mask (kk<=qq); [:, 1, :] = window
            # edge mask (kk>qq) — applied as one two-region strided op
            m04t = pp.tile([128, 2, 128], bf16, tag="m04")
            nc.sync.dma_start(
                m04t[:], m04_d[:].rearrange("p (a b) -> p a b", a=2)
            )

            qk = [
                pp.tile([128, S], bf16, tag=f"qk{c}", name=f"qk{c}") for c in range(4)
            ]
            # per-head K tiles, zero-padded to K=128 (rows 64-127 = 0) so
            # score matmuls are full-partition and pipeline at N-cycle
            # rate; the other head's q rows are nulled by the zero k rows.
            kp = [
                pp.tile([128, S], bf16, tag=f"kp{h}", name=f"kp{h}")
                for h in range(NH)
            ]
            # the k data sits in the same partition half as the head's q
            # rows (even: 0-63, odd: 64-127); the other half is zero
            for h in range(NH):
                hb = (h % 2) * 64
                nc.vector.memset(kp[h][64 - hb : 128 - hb, :], 0.0)
            CV = NH * 65  # 260: per head 64 v-cols + 1 ones col
            vbig = pp.tile([128, NB, CV], bf16, tag="vbig")
            outH = pp.tile([128, 2, S], bf16, tag="outH")
            wpt = pp.tile([128, 2, D], bf16, tag="wpt")

            wbig = xw_pool.tile([128, KT, 3 * NH * HD], bf16, tag="wbig")
            xbig = xw_pool.tile([128, KT, S], bf16, tag="xbig")
            cos2 = xw_pool.tile([128, S], bf16, tag="cos2")
            sin2 = xw_pool.tile([128, S], bf16, tag="sin2")

            # DMA order = need order: v-cols, x first chunk, qk-cols, rope
            # tables, rest of x, c_proj weights.
            for kt in range(KT):
                nc.sync.dma_start(wbig[:, kt, :], wqkv_d[ts(kt, 128), :])
            for kt in range(KT):
                nc.sync.dma_start(xbig[:, kt, 0:512], xT_d[ts(kt, 128), 0:512])
            nc.sync.dma_start(cos2[:], cos2_d[:])
            nc.sync.dma_start(sin2[:], sin2_d[:])
            for kt in range(KT):
                nc.sync.dma_start(xbig[:, kt, 512:1024], xT_d[ts(kt, 128), 512:1024])
            for kt in range(KT):
                nc.sync.dma_start(
                    xbig[:, kt, 1024:2048], xT_d[ts(kt, 128), 1024:2048]
                )
            for k2 in range(2):
                nc.sync.dma_start(wpt[:, k2, :], wp_d[ts(k2, 128), :])

            # PE prewarm: keep the tensor engine busy (and the HAM
            # clock-gate open) while the first input DMAs land.
            for w in range(32):
                psw = psj_pool.tile([128, 512], f32, tag="psj", name="psw")
                nc.tensor.matmul(
                    psw[:], zb[:, 0:128], zb[:, 0:512],
                    start=True, stop=True,
                )

            # ---------------- building blocks ----------------
            eTs = [dict() for _ in range(NH)]  # [h][kj] -> masked exp'd scores

            def proj_v(sb):
                vsb = vbig[:, sb, :].rearrange("p (h c) -> p h c", c=65)
                nc.vector.memset(vsb[:, :, 64:65], 1.0)
                psv = psj_pool.tile([128, 256], f32, tag="psj", name="psv")
                for kt in range(KT):
                    nc.tensor.matmul(
                        psv[:],
                        xbig[:, kt, ts(sb, 128)],
                        wbig[:, kt, ds(2 * NH * HD, NH * HD)],
                        start=(kt == 0),
                        stop=(kt == KT - 1),
                    )
                # alternate evac engine so neither ACT nor DVE builds a
                # backlog in front of the psum pool recycle
                if sb % 2 == 0:
                    nc.vector.tensor_copy(
                        vsb[:, :, 0:64],
                        psv[:].rearrange("p (h c) -> p h c", c=64),
                    )
                else:
                    nc.scalar.copy(
                        vsb[:, :, 0:64],
                        psv[:].rearrange("p (h c) -> p h c", c=64),
                    )

            def proj_qk(c, sc):
                psb = psj_pool.tile([128, 512], f32, tag="psj", name="psb")
                for kt in range(KT):
                    nc.tensor.matmul(
                        psb[:],
                        wbig[:, kt, ts(c, 128)],
                        xbig[:, kt, ts(sc, 512)],
                        start=(kt == 0),
                        stop=(kt == KT - 1),
                    )
                nc.scalar.add(
                    qk[c][:, ts(sc, 512)], psb[:], bqk_t[:, c : c + 1]
                )
                # rope: rotate_half via partition-swap SBUF DMAs on the
                # sync/gpsimd queues (sign is folded into the sin table)
                qc = qk[c][:, ts(sc, 512)]
                tmp = tmp_pool.tile([128, 512], bf16, tag="ropetmp", name="tmp")
                dma_engs = [nc.sync, nc.gpsimd, nc.sync, nc.gpsimd]
                for g in range(2):
                    b0 = g * 64
                    dma_engs[2 * g].dma_start(
                        tmp[b0 : b0 + 32, :],
                        qk[c][b0 + 32 : b0 + 64, ts(sc, 512)],
                    )
                    dma_engs[2 * g + 1].dma_start(
                        tmp[b0 + 32 : b0 + 64, :],
                        qk[c][b0 : b0 + 32, ts(sc, 512)],
                    )
                nc.vector.tensor_tensor(
                    tmp[:], tmp[:], sin2[:, ts(sc, 512)], op=mult
                )
                nc.vector.tensor_tensor(qc, qc, cos2[:, ts(sc, 512)], op=mult)
                nc.vector.tensor_add(qc, qc, tmp[:])
                if c >= 2:
                    # spread each roped k head into its padded K=128 tile
                    # (partition-aligned with the head's q rows)
                    h0 = 2 * (c - 2)
                    nc.sync.dma_start(
                        kp[h0][0:64, ts(sc, 512)], qk[c][0:64, ts(sc, 512)]
                    )
                    nc.gpsimd.dma_start(
                        kp[h0 + 1][64:128, ts(sc, 512)], qk[c][64:128, ts(sc, 512)]
                    )

            def scores_mm(h, kj, pss, nq, part):
                # one scores matmul sT[k, q]; part 0 = first 512 q-cols,
                # part 1 = the 128-col band tail. lhsT is the head's
                # zero-padded K=128 tile; the moving q carries both heads'
                # rows (the foreign head is nulled by the zero k rows).
                qt = qk[h // 2]
                n1 = min(512, nq * 128)
                n2 = nq * 128 - n1
                lhs_k = kp[h][:, ts(kj, 128)]
                if part == 0:
                    nc.tensor.matmul(
                        pss[:, 0:n1],
                        lhs_k,
                        qt[:, ds(kj * 128, n1)],
                        start=True,
                        stop=True,
                    )
                elif n2:
                    nc.tensor.matmul(
                        pss[:, 512 : 512 + n2],
                        lhs_k,
                        qt[:, ds(kj * 128 + 512, n2)],
                        start=True,
                        stop=True,
                    )

            def exp_mask(h, kj, pss, nq):
                eT = e_pool.tile([128, 640], bf16, tag="et", name="eT")
                nc.scalar.activation(
                    eT[:, 0 : nq * 128], pss[:, 0 : nq * 128], Exp, scale=0.125
                )
                # banded mask: diag block keeps kk<=qq, window edge kk>qq.
                # Both 128-col regions are masked in one strided DVE op.
                if nq == WB + 1:
                    e2 = eT[:, 0:640].rearrange("p (x y) -> p x y", y=128)[
                        :, 0:5:4, :
                    ]
                    nc.vector.tensor_tensor(e2, e2, m04t[:], op=mult)
                else:
                    nc.vector.tensor_tensor(
                        eT[:, 0:128], eT[:, 0:128], m04t[:, 0, :], op=mult
                    )
                eTs[h][kj] = eT

            def evac_q(po, h, qtr):
                # normalize by denominators (PSUM row 64) into outH.
                # The denom row is staged via ACT while ACT has slack
                # (early waves); via DVE once ACT is exp-bound late.
                hb = (h % 2) * 64
                rb = rb_pool.tile([64, HS], f32, tag="rb", name="rb")
                if qtr == 2:
                    nc.vector.tensor_copy(rb[0:1, :], po[64:65, :])
                else:
                    nc.scalar.copy(rb[0:1, :], po[64:65, :])
                nc.vector.reciprocal_approx_fast(rb[0:1, :], rb[0:1, :])
                nc.gpsimd.partition_broadcast(rb[:], rb[0:1, :])
                nc.vector.tensor_tensor(
                    outH[hb : hb + 64, h // 2, qtr * HS : (qtr + 1) * HS],
                    po[0:64, :],
                    rb[:],
                    op=mult,
                )

            def attnv(h, qtr):
                # kj-major attn@V: each V block loads once and streams its
                # whole q-span (per-element has_written bits handle the
                # staggered accumulation regions)
                kjlo = max(0, 4 * qtr - WB)
                kjhi = 4 * qtr + QB - 1
                po = pso_pool.tile([65, HS], f32, tag="pso", name="po")
                for kj in range(kjlo, kjhi + 1):
                    qlo = max(4 * qtr, kj)
                    qhi = min(4 * qtr + QB - 1, kj + WB)
                    off = (qlo - kj) * 128
                    n = (qhi - qlo + 1) * 128
                    nc.tensor.matmul(
                        po[:, ds((qlo - 4 * qtr) * 128, n)],
                        vbig[:, kj, h * 65 : h * 65 + 65],
                        eTs[h][kj][:, off : off + n],
                        start=(kj == kjlo),
                        stop=(kj == kjhi),
                    )
                evac_q(po, h, qtr)

            def cproj(sb):
                # two 1-bank psum groups so c_proj never contends with the
                # scores pool
                yt = y_pool.tile([128, D], bf16, tag="yo", name="yt")
                for ncol in range(2):
                    psp = psj_pool.tile([128, 512], f32, tag="psj", name="psp")
                    for k2 in range(2):
                        nc.tensor.matmul(
                            psp[:],
                            outH[:, k2, ts(sb, 128)],
                            wpt[:, k2, ts(ncol, 512)],
                            start=(k2 == 0),
                            stop=(k2 == 1),
                        )
                    if (sb + ncol) % 2 == 0:
                        nc.scalar.copy(yt[:, ts(ncol, 512)], psp[:])
                    else:
                        nc.vector.tensor_copy(yt[:, ts(ncol, 512)], psp[:])
                nc.sync.dma_start(out_d[ts(sb, 128), :], yt[:])

            def scores_kj(kj):
                # scores+exp for one key block, head-pair interleaved
                nq = min(WB + 1, NB - kj)
                for pair in range(2):
                    h0, h1 = 2 * pair, 2 * pair + 1
                    p0 = ps_pool.tile([128, 640], f32, tag="ps", name="pss")
                    scores_mm(h0, kj, p0, nq, 0)
                    scores_mm(h0, kj, p0, nq, 1)
                    p1 = ps_pool.tile([128, 640], f32, tag="ps", name="pss")
                    scores_mm(h1, kj, p1, nq, 0)
                    scores_mm(h1, kj, p1, nq, 1)
                    exp_mask(h0, kj, p0, nq)
                    exp_mask(h1, kj, p1, nq)

            def wave(qtr):
                # Scores run a full wave ahead: this wave consumes quarter
                # qtr's eT tiles (exp'd during wave qtr-1) and produces
                # quarter qtr+1's. Projection chunks lead the wave so the
                # PE stays dense (and the clock-gate warm) while ACT drains
                # the exp backlog; attn@V heads, new score blocks, and the
                # previous quarter's c_proj interleave so no engine queue
                # builds a deep backlog in front of a dependency.
                ch = qtr + 2
                if ch < 4:
                    for j in range(4):
                        proj_v(4 * ch + j)
                        proj_qk((0, 2, 1, 3)[j], ch)
                # c_proj of the previous quarter starts one slot late so
                # its first group never waits on that quarter's last
                # normalization chain
                for i in range(4):
                    if qtr >= 1 and i >= 1:
                        cproj(4 * (qtr - 1) + i - 1)
                    attnv(i, qtr)
                    if qtr < 3:
                        scores_kj(4 * qtr + 4 + i)
                if qtr >= 1:
                    cproj(4 * (qtr - 1) + 3)

            # ---------------- pipelined schedule ----------------
            for sc in range(2):
                for sb in range(4 * sc, 4 * sc + 4):
                    proj_v(sb)
                for c in (0, 2, 1, 3):
                    proj_qk(c, sc)
            for kj in range(4):
                scores_kj(kj)
            for qtr in range(4):
                wave(qtr)
            for sb in range(12, 16):
                cproj(sb)

    nc.compile()
    return nc


def _host_inputs(hidden, pos, caw, cab, cpw):
    """Build the 8 per-core input maps."""
    inv = 1.0 / (ROPE_BASE ** (np.arange(0, HD, 2, dtype=np.float32) / HD))
    t = np.arange(S, dtype=np.float32)
    freqs = np.outer(t, inv).astype(np.float32)
    emb = np.concatenate([freqs, freqs], axis=1)  # [S, HD]
    cos = np.cos(emb).astype(np.float32)
    sin = np.sin(emb).astype(np.float32)

    import ml_dtypes

    bf = ml_dtypes.bfloat16
    ii = np.arange(128)
    m0 = (ii[:, None] <= ii[None, :]).astype(bf)
    m4 = (ii[:, None] > ii[None, :]).astype(bf)
    m04 = np.ascontiguousarray(np.concatenate([m0, m4], axis=1))

    xTs, cos2s, sin2s = [], [], []
    for b in range(B):
        xTs.append(np.ascontiguousarray(hidden[b].T).astype(bf))
        cosT = np.ascontiguousarray(cos[pos[b]].T)  # [HD, S]
        sinT = np.ascontiguousarray(sin[pos[b]].T)
        sinS = np.concatenate([-sinT[:32], sinT[32:]], axis=0)
        cos2s.append(np.tile(cosT, (2, 1)).astype(bf))
        sin2s.append(np.tile(sinS, (2, 1)).astype(bf))

    in_maps = []
    for c in range(NCORES):
        b = c // 4
        h0 = NH * (c % 4)
        col = h0 * HD
        w_q = caw[:, col : col + NH * HD]
        w_k = caw[:, D + col : D + col + NH * HD]
        w_v = caw[:, 2 * D + col : 2 * D + col + NH * HD]
        wqkv = np.ascontiguousarray(
            np.concatenate([w_q, w_k, w_v], axis=1)
        ).astype(bf)
        b_q = cab[col : col + NH * HD]
        b_k = cab[D + col : D + col + NH * HD]
        bqk = np.ascontiguousarray(
            np.concatenate([b_q, b_k]).reshape(4, 128).T
        )  # [128, 4]: partition = col within tile
        wp = np.ascontiguousarray(cpw[col : col + NH * HD, :]).astype(bf)
        in_maps.append(
            {
                "xT": xTs[b],
                "wqkv": wqkv,
                "bqk": bqk,
                "wp": wp,
                "cos2": cos2s[b],
                "sin2": sin2s[b],
                "m04": m04,
            }
        )
    return in_maps


def _assemble(results, cab, cpw, cpb):
    """Host all-reduce of the 4 per-batch partials + biases.

    The V-bias contribution is position-independent after softmax
    (attn rows sum to 1), so it folds into a constant row vector:
    bias_v @ c_proj_w.
    """
    vrow = cab[2 * D :].astype(np.float32) @ cpw.astype(np.float32)
    bias = cpb.astype(np.float32) + vrow
    y = np.empty((B, S, D), dtype=np.float32)
    for b in range(B):
        acc = results[4 * b]["out"].astype(np.float32)
        for c in range(4 * b + 1, 4 * b + 4):
            acc = acc + results[c]["out"].astype(np.float32)
        y[b] = acc + bias[None, :]
    return y


def kernel(**inputs):
    from concourse import bass_utils

    hidden = np.asarray(inputs["hidden_states"], dtype=np.float32)
    pos = np.asarray(inputs["position_ids"]).astype(np.int64)
    caw = np.asarray(inputs["c_attn_w"], dtype=np.float32)
    cab = np.asarray(inputs["c_attn_b"], dtype=np.float32)
    cpw = np.asarray(inputs["c_proj_w"], dtype=np.float32)
    cpb = np.asarray(inputs["c_proj_b"], dtype=np.float32)

    in_maps = _host_inputs(hidden, pos, caw, cab, cpw)
    nc = _build_nc()
    res = bass_utils.run_bass_kernel_spmd(nc, in_maps, list(range(NCORES)))
    return _assemble(res.results, cab, cpw, cpb)


# revision 50
# speedup vs baseline: 1.0410x; 1.0285x over previous
"""Trainium2 Bass kernel: GPT2-style windowed attention (DecisionTransformer).

Full-input contract: kernel(**inputs) -> [B, S, D] float32.

Sharding: batch*heads across 8 cores (core c -> batch c//4, heads 4*(c%4)..+4).
Each core: column-sliced c_attn, full windowed attention for its 4 heads,
row-sliced c_proj producing a partial [S, D] output; host sums partials
(the "all-reduce") and adds c_proj bias + V-bias contribution once.

Layout / schedule choices:
  - all matmul operands are bf16 (tolerance is 2e-2; bf16 keeps PE at
    1 cyc/row and halves HBM traffic). PSUM accumulation stays fp32.
  - hidden is sent pre-transposed (xT [D, S]) so QK projections emit
    qT/kT directly in [head*dim, seq] layout; V is projected in
    [seq, head*dim] layout with a ones-column per head so attn@V
    accumulates softmax denominators in PSUM row 64 for free. V-bias
    folds into the host-side reduce (softmax rows sum to 1).
  - the whole kernel is software-pipelined: projections advance per
    512-col seq chunk, and attention q-quarters (scores+exp for all 4
    heads, kj-major attn@V, normalize, c_proj slice, output DMA) are
    issued as soon as their chunk dependencies are met. ACT does only
    exp during attention (the gating engine); evacuations go to DVE.
  - rope: rotate_half via 4 cross-quadrant 32-partition copies (2 DVE,
    2 GPSIMD) -- no DMA, no partition-swap latency chain.
  - scores for a head pair (partitions 0-63 / 64-127, K=64 each) are
    issued back-to-back so the PE runs them as concurrent row-tiles.
  - a dozen dummy matmuls at t=0 keep the PE busy (and the HAM
    clock-gate open) while the first input DMAs land.
"""

import sys

import numpy as np

sys.path.insert(0, "/opt/trn_rl_repo")

B, S, D = 2, 2048, 1024
H, HD = 16, 64
WINDOW = 512
ROPE_BASE = 4000.0
NCORES = 8
NH = 4          # heads per core
KT = D // 128   # 8 contraction tiles for c_attn
NB = S // 128   # 16 seq blocks
WB = WINDOW // 128  # 4 -> band spans up to 5 query blocks per key block


def _build_nc(debug_taps=False):
    import concourse.bass as bass
    from concourse import bacc, library_config, mybir
    import concourse.tile as tile

    f32 = mybir.dt.float32
    bf16 = mybir.dt.bfloat16
    Exp = mybir.ActivationFunctionType.Exp
    mult = mybir.AluOpType.mult
    ts = bass.ts
    ds = bass.ds

    nc = bacc.Bacc("TRN2")

    xT_d = nc.dram_tensor("xT", [D, S], bf16, kind="ExternalInput")
    wqkv_d = nc.dram_tensor("wqkv", [D, 3 * NH * HD], bf16, kind="ExternalInput")
    bqk_d = nc.dram_tensor("bqk", [128, 4], f32, kind="ExternalInput")
    wp_d = nc.dram_tensor("wp", [NH * HD, D], bf16, kind="ExternalInput")
    cos2_d = nc.dram_tensor("cos2", [128, S], bf16, kind="ExternalInput")
    sin2_d = nc.dram_tensor("sin2", [128, S], bf16, kind="ExternalInput")
    m04_d = nc.dram_tensor("m04", [128, 256], bf16, kind="ExternalInput")
    out_d = nc.dram_tensor("out", [S, D], bf16, kind="ExternalOutput")

    HS = S // 4  # q-quarter span: po is 1 PSUM bank
    QB = NB // 4  # 4 q-blocks per quarter

    with tile.TileContext(nc) as tc:
        nc.gpsimd.load_library(library_config.attn)

        with (
            tc.tile_pool(name="persist", bufs=1) as pp,
            tc.tile_pool(name="psj", bufs=2, space="PSUM") as psj_pool,
            tc.tile_pool(name="ps", bufs=2, space="PSUM") as ps_pool,
            tc.tile_pool(name="pso", bufs=2, space="PSUM") as pso_pool,
            tc.tile_pool(name="xw", bufs=1) as xw_pool,
            tc.tile_pool(name="ropetmp", bufs=6) as tmp_pool,
            tc.tile_pool(name="et", bufs=40) as e_pool,
            tc.tile_pool(name="rb", bufs=4) as rb_pool,
            tc.tile_pool(name="yo", bufs=4) as y_pool,
        ):
            # prewarm tile: zeros, matmul'd while input DMAs land
            zb = pp.tile([128, 512], bf16, tag="zb")
            nc.vector.memset(zb[:], 0.0)

            bqk_t = pp.tile([128, 4], f32, tag="bqk")
            nc.sync.dma_start(bqk_t[:], bqk_d[:])
            # m04[:, 0, :] = diag-block mask (kk<=qq); [:, 1, :] = window
            # edge mask (kk>qq) — applied as one two-region strided op
            m04t = pp.tile([128, 2, 128], bf16, tag="m04")
            nc.sync.dma_start(
                m04t[:], m04_d[:].rearrange("p (a b) -> p a b", a=2)
            )

            qk = [
                pp.tile([128, S], bf16, tag=f"qk{c}", name=f"qk{c}") for c in range(4)
            ]
            # per-head K tiles, zero-padded to K=128 (rows 64-127 = 0) so
            # score matmuls are full-partition and pipeline at N-cycle
            # rate; the other head's q rows are nulled by the zero k rows.
            kp = [
                pp.tile([128, S], bf16, tag=f"kp{h}", name=f"kp{h}")
                for h in range(NH)
            ]
            # the k data sits in the same partition half as the head's q
            # rows (even: 0-63, odd: 64-127); the other half is zero
            for h in range(NH):
                hb = (h % 2) * 64
                nc.vector.memset(kp[h][64 - hb : 128 - hb, :], 0.0)
            CV = NH * 65  # 260: per head 64 v-cols + 1 ones col
            vbig = pp.tile([128, NB, CV], bf16, tag="vbig")
            outH = pp.tile([128, 2, S], bf16, tag="outH")
            wpt = pp.tile([128, 2, D], bf16, tag="wpt")

            wbig = xw_pool.tile([128, KT, 3 * NH * HD], bf16, tag="wbig")
            xbig = xw_pool.tile([128, KT, S], bf16, tag="xbig")
            cos2 = xw_pool.tile([128, S], bf16, tag="cos2")
            sin2 = xw_pool.tile([128, S], bf16, tag="sin2")

            # DMA order = need order: v-cols, x first chunk, qk-cols, rope
            # tables, rest of x, c_proj weights.
            for kt in range(KT):
                nc.sync.dma_start(wbig[:, kt, :], wqkv_d[ts(kt, 128), :])
            for kt in range(KT):
                nc.sync.dma_start(xbig[:, kt, 0:512], xT_d[ts(kt, 128), 0:512])
            nc.sync.dma_start(cos2[:], cos2_d[:])
            nc.sync.dma_start(sin2[:], sin2_d[:])
            for kt in range(KT):
                nc.sync.dma_start(xbig[:, kt, 512:1024], xT_d[ts(kt, 128), 512:1024])
            for kt in range(KT):
                nc.sync.dma_start(
                    xbig[:, kt, 1024:2048], xT_d[ts(kt, 128), 1024:2048]
                )
            for k2 in range(2):
                nc.sync.dma_start(wpt[:, k2, :], wp_d[ts(k2, 128), :])

            # PE prewarm: keep the tensor engine busy (and the HAM
            # clock-gate open) while the first input DMAs land.
            for w in range(32):
                psw = psj_pool.tile([128, 512], f32, tag="psj", name="psw")
                nc.tensor.matmul(
                    psw[:], zb[:, 0:128], zb[:, 0:512],
                    start=True, stop=True,
                )

            # ---------------- building blocks ----------------
            eTs = [dict() for _ in range(NH)]  # [h][kj] -> masked exp'd scores

            def proj_v(sb):
                vsb = vbig[:, sb, :].rearrange("p (h c) -> p h c", c=65)
                nc.vector.memset(vsb[:, :, 64:65], 1.0)
                psv = psj_pool.tile([128, 256], f32, tag="psj", name="psv")
                for kt in range(KT):
                    nc.tensor.matmul(
                        psv[:],
                        xbig[:, kt, ts(sb, 128)],
                        wbig[:, kt, ds(2 * NH * HD, NH * HD)],
                        start=(kt == 0),
                        stop=(kt == KT - 1),
                    )
                nc.vector.tensor_copy(
                    vsb[:, :, 0:64],
                    psv[:].rearrange("p (h c) -> p h c", c=64),
                )

            def proj_qk(c, sc):
                psb = psj_pool.tile([128, 512], f32, tag="psj", name="psb")
                for kt in range(KT):
                    nc.tensor.matmul(
                        psb[:],
                        wbig[:, kt, ts(c, 128)],
                        xbig[:, kt, ts(sc, 512)],
                        start=(kt == 0),
                        stop=(kt == KT - 1),
                    )
                nc.scalar.add(
                    qk[c][:, ts(sc, 512)], psb[:], bqk_t[:, c : c + 1]
                )
                # rope: rotate_half via partition-swap SBUF DMAs on the
                # sync/gpsimd queues (sign is folded into the sin table)
                qc = qk[c][:, ts(sc, 512)]
                tmp = tmp_pool.tile([128, 512], bf16, tag="ropetmp", name="tmp")
                dma_engs = [nc.sync, nc.gpsimd, nc.sync, nc.gpsimd]
                for g in range(2):
                    b0 = g * 64
                    dma_engs[2 * g].dma_start(
                        tmp[b0 : b0 + 32, :],
                        qk[c][b0 + 32 : b0 + 64, ts(sc, 512)],
                    )
                    dma_engs[2 * g + 1].dma_start(
                        tmp[b0 + 32 : b0 + 64, :],
                        qk[c][b0 : b0 + 32, ts(sc, 512)],
                    )
                nc.vector.tensor_tensor(
                    tmp[:], tmp[:], sin2[:, ts(sc, 512)], op=mult
                )
                nc.vector.tensor_tensor(qc, qc, cos2[:, ts(sc, 512)], op=mult)
                nc.vector.tensor_add(qc, qc, tmp[:])
                if c >= 2:
                    # spread each roped k head into its padded K=128 tile
                    # (partition-aligned with the head's q rows)
                    h0 = 2 * (c - 2)
                    nc.sync.dma_start(
                        kp[h0][0:64, ts(sc, 512)], qk[c][0:64, ts(sc, 512)]
                    )
                    nc.gpsimd.dma_start(
                        kp[h0 + 1][64:128, ts(sc, 512)], qk[c][64:128, ts(sc, 512)]
                    )

            def scores_mm(h, kj, pss, nq, part):
                # one scores matmul sT[k, q]; part 0 = first 512 q-cols,
                # part 1 = the 128-col band tail. lhsT is the head's
                # zero-padded K=128 tile; the moving q carries both heads'
                # rows (the foreign head is nulled by the zero k rows).
                qt = qk[h // 2]
                n1 = min(512, nq * 128)
                n2 = nq * 128 - n1
                lhs_k = kp[h][:, ts(kj, 128)]
                if part == 0:
                    nc.tensor.matmul(
                        pss[:, 0:n1],
                        lhs_k,
                        qt[:, ds(kj * 128, n1)],
                        start=True,
                        stop=True,
                    )
                elif n2:
                    nc.tensor.matmul(
                        pss[:, 512 : 512 + n2],
                        lhs_k,
                        qt[:, ds(kj * 128 + 512, n2)],
                        start=True,
                        stop=True,
                    )

            def exp_mask(h, kj, pss, nq):
                eT = e_pool.tile([128, 640], bf16, tag="et", name="eT")
                nc.scalar.activation(
                    eT[:, 0 : nq * 128], pss[:, 0 : nq * 128], Exp, scale=0.125
                )
                # banded mask: diag block keeps kk<=qq, window edge kk>qq.
                # Both 128-col regions are masked in one strided DVE op.
                if nq == WB + 1:
                    e2 = eT[:, 0:640].rearrange("p (x y) -> p x y", y=128)[
                        :, 0:5:4, :
                    ]
                    nc.vector.tensor_tensor(e2, e2, m04t[:], op=mult)
                else:
                    nc.vector.tensor_tensor(
                        eT[:, 0:128], eT[:, 0:128], m04t[:, 0, :], op=mult
                    )
                eTs[h][kj] = eT

            def evac_q(po, h, qtr):
                # normalize by denominators (PSUM row 64) into outH.
                # The denom row is staged via ACT while ACT has slack
                # (early waves); via DVE once ACT is exp-bound late.
                hb = (h % 2) * 64
                rb = rb_pool.tile([64, HS], f32, tag="rb", name="rb")
                if qtr == 2:
                    nc.vector.tensor_copy(rb[0:1, :], po[64:65, :])
                else:
                    nc.scalar.copy(rb[0:1, :], po[64:65, :])
                nc.vector.reciprocal_approx_fast(rb[0:1, :], rb[0:1, :])
                nc.gpsimd.partition_broadcast(rb[:], rb[0:1, :])
                nc.vector.tensor_tensor(
                    outH[hb : hb + 64, h // 2, qtr * HS : (qtr + 1) * HS],
                    po[0:64, :],
                    rb[:],
                    op=mult,
                )

            def attnv(h, qtr):
                # kj-major attn@V: each V block loads once and streams its
                # whole q-span (per-element has_written bits handle the
                # staggered accumulation regions)
                kjlo = max(0, 4 * qtr - WB)
                kjhi = 4 * qtr + QB - 1
                po = pso_pool.tile([65, HS], f32, tag="pso", name="po")
                for kj in range(kjlo, kjhi + 1):
                    qlo = max(4 * qtr, kj)
                    qhi = min(4 * qtr + QB - 1, kj + WB)
                    off = (qlo - kj) * 128
                    n = (qhi - qlo + 1) * 128
                    nc.tensor.matmul(
                        po[:, ds((qlo - 4 * qtr) * 128, n)],
                        vbig[:, kj, h * 65 : h * 65 + 65],
                        eTs[h][kj][:, off : off + n],
                        start=(kj == kjlo),
                        stop=(kj == kjhi),
                    )
                evac_q(po, h, qtr)

            def cproj(sb):
                # two 1-bank psum groups so c_proj never contends with the
                # scores pool
                yt = y_pool.tile([128, D], bf16, tag="yo", name="yt")
                for ncol in range(2):
                    psp = psj_pool.tile([128, 512], f32, tag="psj", name="psp")
                    for k2 in range(2):
                        nc.tensor.matmul(
                            psp[:],
                            outH[:, k2, ts(sb, 128)],
                            wpt[:, k2, ts(ncol, 512)],
                            start=(k2 == 0),
                            stop=(k2 == 1),
                        )
                    if (sb + ncol) % 2 == 0:
                        nc.scalar.copy(yt[:, ts(ncol, 512)], psp[:])
                    else:
                        nc.vector.tensor_copy(yt[:, ts(ncol, 512)], psp[:])
                    # ship each half as soon as its evacuation lands
                    nc.sync.dma_start(
                        out_d[ts(sb, 128), ts(ncol, 512)], yt[:, ts(ncol, 512)]
                    )

            def scores_kj(kj):
                # scores+exp for one key block, head-pair interleaved
                nq = min(WB + 1, NB - kj)
                for pair in range(2):
                    h0, h1 = 2 * pair, 2 * pair + 1
                    p0 = ps_pool.tile([128, 640], f32, tag="ps", name="pss")
                    scores_mm(h0, kj, p0, nq, 0)
                    scores_mm(h0, kj, p0, nq, 1)
                    p1 = ps_pool.tile([128, 640], f32, tag="ps", name="pss")
                    scores_mm(h1, kj, p1, nq, 0)
                    scores_mm(h1, kj, p1, nq, 1)
                    exp_mask(h0, kj, p0, nq)
                    exp_mask(h1, kj, p1, nq)

            def wave(qtr):
                # Scores run a full wave ahead: this wave consumes quarter
                # qtr's eT tiles (exp'd during wave qtr-1) and produces
                # quarter qtr+1's. Projection chunks lead the wave so the
                # PE stays dense (and the clock-gate warm) while ACT drains
                # the exp backlog; attn@V heads, new score blocks, and the
                # previous quarter's c_proj interleave so no engine queue
                # builds a deep backlog in front of a dependency.
                ch = qtr + 2
                if ch < 4:
                    for j in range(4):
                        proj_v(4 * ch + j)
                        proj_qk((0, 2, 1, 3)[j], ch)
                # c_proj of the previous quarter starts one slot late so
                # its first group never waits on that quarter's last
                # normalization chain
                for i in range(4):
                    if qtr >= 1 and i >= 1:
                        cproj(4 * (qtr - 1) + i - 1)
                    attnv(i, qtr)
                    if qtr < 3:
                        scores_kj(4 * qtr + 4 + i)
                if qtr >= 1:
                    cproj(4 * (qtr - 1) + 3)

            # ---------------- pipelined schedule ----------------
            for sc in range(2):
                for sb in range(4 * sc, 4 * sc + 4):
                    proj_v(sb)
                for c in (0, 2, 1, 3):
                    proj_qk(c, sc)
            for kj in range(4):
                scores_kj(kj)
            for qtr in range(4):
                wave(qtr)
            for sb in range(12, 16):
                cproj(sb)

    nc.compile()
    return nc


def _host_inputs(hidden, pos, caw, cab, cpw):
    """Build the 8 per-core input maps."""
    inv = 1.0 / (ROPE_BASE ** (np.arange(0, HD, 2, dtype=np.float32) / HD))
    t = np.arange(S, dtype=np.float32)
    freqs = np.outer(t, inv).astype(np.float32)
    emb = np.concatenate([freqs, freqs], axis=1)  # [S, HD]
    cos = np.cos(emb).astype(np.float32)
    sin = np.sin(emb).astype(np.float32)

    import ml_dtypes

    bf = ml_dtypes.bfloat16
    ii = np.arange(128)
    m0 = (ii[:, None] <= ii[None, :]).astype(bf)
    m4 = (ii[:, None] > ii[None, :]).astype(bf)
    m04 = np.ascontiguousarray(np.concatenate([m0, m4], axis=1))

    xTs, cos2s, sin2s = [], [], []
    for b in range(B):
        xTs.append(np.ascontiguousarray(hidden[b].T).astype(bf))
        cosT = np.ascontiguousarray(cos[pos[b]].T)  # [HD, S]
        sinT = np.ascontiguousarray(sin[pos[b]].T)
        sinS = np.concatenate([-sinT[:32], sinT[32:]], axis=0)
        cos2s.append(np.tile(cosT, (2, 1)).astype(bf))
        sin2s.append(np.tile(sinS, (2, 1)).astype(bf))

    in_maps = []
    for c in range(NCORES):
        b = c // 4
        h0 = NH * (c % 4)
        col = h0 * HD
        w_q = caw[:, col : col + NH * HD]
        w_k = caw[:, D + col : D + col + NH * HD]
        w_v = caw[:, 2 * D + col : 2 * D + col + NH * HD]
        wqkv = np.ascontiguousarray(
            np.concatenate([w_q, w_k, w_v], axis=1)
        ).astype(bf)
        b_q = cab[col : col + NH * HD]
        b_k = cab[D + col : D + col + NH * HD]
        bqk = np.ascontiguousarray(
            np.concatenate([b_q, b_k]).reshape(4, 128).T
        )  # [128, 4]: partition = col within tile
        wp = np.ascontiguousarray(cpw[col : col + NH * HD, :]).astype(bf)
        in_maps.append(
            {
                "xT": xTs[b],
                "wqkv": wqkv,
                "bqk": bqk,
                "wp": wp,
                "cos2": cos2s[b],
                "sin2": sin2s[b],
                "m04": m04,
            }
        )
    return in_maps


def _assemble(results, cab, cpw, cpb):
    """Host all-reduce of the 4 per-batch partials + biases.

    The V-bias contribution is position-independent after softmax
    (attn rows sum to 1), so it folds into a constant row vector:
    bias_v @ c_proj_w.
    """
    vrow = cab[2 * D :].astype(np.float32) @ cpw.astype(np.float32)
    bias = cpb.astype(np.float32) + vrow
    y = np.empty((B, S, D), dtype=np.float32)
    for b in range(B):
        acc = results[4 * b]["out"].astype(np.float32)
        for c in range(4 * b + 1, 4 * b + 4):
            acc = acc + results[c]["out"].astype(np.float32)
        y[b] = acc + bias[None, :]
    return y


def kernel(**inputs):
    from concourse import bass_utils

    hidden = np.asarray(inputs["hidden_states"], dtype=np.float32)
    pos = np.asarray(inputs["position_ids"]).astype(np.int64)
    caw = np.asarray(inputs["c_attn_w"], dtype=np.float32)
    cab = np.asarray(inputs["c_attn_b"], dtype=np.float32)
    cpw = np.asarray(inputs["c_proj_w"], dtype=np.float32)
    cpb = np.asarray(inputs["c_proj_b"], dtype=np.float32)

    in_maps = _host_inputs(hidden, pos, caw, cab, cpw)
    nc = _build_nc()
    res = bass_utils.run_bass_kernel_spmd(nc, in_maps, list(range(NCORES)))
    return _assemble(res.results, cab, cpw, cpb)


# revision 51
# speedup vs baseline: 1.0442x; 1.0031x over previous
"""Trainium2 Bass kernel: GPT2-style windowed attention (DecisionTransformer).

Full-input contract: kernel(**inputs) -> [B, S, D] float32.

Sharding: batch*heads across 8 cores (core c -> batch c//4, heads 4*(c%4)..+4).
Each core: column-sliced c_attn, full windowed attention for its 4 heads,
row-sliced c_proj producing a partial [S, D] output; host sums partials
(the "all-reduce") and adds c_proj bias + V-bias contribution once.

Layout / schedule choices:
  - all matmul operands are bf16 (tolerance is 2e-2; bf16 keeps PE at
    1 cyc/row and halves HBM traffic). PSUM accumulation stays fp32.
  - hidden is sent pre-transposed (xT [D, S]) so QK projections emit
    qT/kT directly in [head*dim, seq] layout; V is projected in
    [seq, head*dim] layout with a ones-column per head so attn@V
    accumulates softmax denominators in PSUM row 64 for free. V-bias
    folds into the host-side reduce (softmax rows sum to 1).
  - the whole kernel is software-pipelined: projections advance per
    512-col seq chunk, and attention q-quarters (scores+exp for all 4
    heads, kj-major attn@V, normalize, c_proj slice, output DMA) are
    issued as soon as their chunk dependencies are met. ACT does only
    exp during attention (the gating engine); evacuations go to DVE.
  - rope: rotate_half via 4 cross-quadrant 32-partition copies (2 DVE,
    2 GPSIMD) -- no DMA, no partition-swap latency chain.
  - scores for a head pair (partitions 0-63 / 64-127, K=64 each) are
    issued back-to-back so the PE runs them as concurrent row-tiles.
  - a dozen dummy matmuls at t=0 keep the PE busy (and the HAM
    clock-gate open) while the first input DMAs land.
"""

import sys

import numpy as np

sys.path.insert(0, "/opt/trn_rl_repo")

B, S, D = 2, 2048, 1024
H, HD = 16, 64
WINDOW = 512
ROPE_BASE = 4000.0
NCORES = 8
NH = 4          # heads per core
KT = D // 128   # 8 contraction tiles for c_attn
NB = S // 128   # 16 seq blocks
WB = WINDOW // 128  # 4 -> band spans up to 5 query blocks per key block


def _build_nc(debug_taps=False):
    import concourse.bass as bass
    from concourse import bacc, library_config, mybir
    import concourse.tile as tile

    f32 = mybir.dt.float32
    bf16 = mybir.dt.bfloat16
    Exp = mybir.ActivationFunctionType.Exp
    mult = mybir.AluOpType.mult
    ts = bass.ts
    ds = bass.ds

    nc = bacc.Bacc("TRN2")

    xT_d = nc.dram_tensor("xT", [D, S], bf16, kind="ExternalInput")
    wqkv_d = nc.dram_tensor("wqkv", [D, 3 * NH * HD], bf16, kind="ExternalInput")
    bqk_d = nc.dram_tensor("bqk", [128, 4], f32, kind="ExternalInput")
    wp_d = nc.dram_tensor("wp", [NH * HD, D], bf16, kind="ExternalInput")
    cos2_d = nc.dram_tensor("cos2", [128, S], bf16, kind="ExternalInput")
    sin2_d = nc.dram_tensor("sin2", [128, S], bf16, kind="ExternalInput")
    m04_d = nc.dram_tensor("m04", [128, 256], bf16, kind="ExternalInput")
    out_d = nc.dram_tensor("out", [S, D], bf16, kind="ExternalOutput")

    HS = S // 4  # q-quarter span: po is 1 PSUM bank
    QB = NB // 4  # 4 q-blocks per quarter

    with tile.TileContext(nc) as tc:
        nc.gpsimd.load_library(library_config.attn)

        with (
            tc.tile_pool(name="persist", bufs=1) as pp,
            tc.tile_pool(name="psj", bufs=2, space="PSUM") as psj_pool,
            tc.tile_pool(name="ps", bufs=2, space="PSUM") as ps_pool,
            tc.tile_pool(name="pso", bufs=2, space="PSUM") as pso_pool,
            tc.tile_pool(name="xw", bufs=1) as xw_pool,
            tc.tile_pool(name="ropetmp", bufs=6) as tmp_pool,
            tc.tile_pool(name="et", bufs=40) as e_pool,
            tc.tile_pool(name="rb", bufs=4) as rb_pool,
            tc.tile_pool(name="yo", bufs=4) as y_pool,
        ):
            # prewarm tile: zeros, matmul'd while input DMAs land
            zb = pp.tile([128, 512], bf16, tag="zb")
            nc.vector.memset(zb[:], 0.0)

            bqk_t = pp.tile([128, 4], f32, tag="bqk")
            nc.sync.dma_start(bqk_t[:], bqk_d[:])
            # m04[:, 0, :] = diag-block mask (kk<=qq); [:, 1, :] = window
            # edge mask (kk>qq) — applied as one two-region strided op
            m04t = pp.tile([128, 2, 128], bf16, tag="m04")
            nc.sync.dma_start(
                m04t[:], m04_d[:].rearrange("p (a b) -> p a b", a=2)
            )

            qk = [
                pp.tile([128, S], bf16, tag=f"qk{c}", name=f"qk{c}") for c in range(4)
            ]
            # per-head K tiles, zero-padded to K=128 (rows 64-127 = 0) so
            # score matmuls are full-partition and pipeline at N-cycle
            # rate; the other head's q rows are nulled by the zero k rows.
            kp = [
                pp.tile([128, S], bf16, tag=f"kp{h}", name=f"kp{h}")
                for h in range(NH)
            ]
            # the k data sits in the same partition half as the head's q
            # rows (even: 0-63, odd: 64-127); the other half is zero
            for h in range(NH):
                hb = (h % 2) * 64
                nc.vector.memset(kp[h][64 - hb : 128 - hb, :], 0.0)
            CV = NH * 65  # 260: per head 64 v-cols + 1 ones col
            vbig = pp.tile([128, NB, CV], bf16, tag="vbig")
            outH = pp.tile([128, 2, S], bf16, tag="outH")
            wpt = pp.tile([128, 2, D], bf16, tag="wpt")

            wbig = xw_pool.tile([128, KT, 3 * NH * HD], bf16, tag="wbig")
            xbig = xw_pool.tile([128, KT, S], bf16, tag="xbig")
            cos2 = xw_pool.tile([128, S], bf16, tag="cos2")
            sin2 = xw_pool.tile([128, S], bf16, tag="sin2")

            # DMA order = need order: v-cols, x first chunk, qk-cols, rope
            # tables, rest of x, c_proj weights.
            for kt in range(KT):
                nc.sync.dma_start(wbig[:, kt, :], wqkv_d[ts(kt, 128), :])
            for kt in range(KT):
                nc.sync.dma_start(xbig[:, kt, 0:512], xT_d[ts(kt, 128), 0:512])
            nc.sync.dma_start(cos2[:], cos2_d[:])
            nc.sync.dma_start(sin2[:], sin2_d[:])
            for kt in range(KT):
                nc.sync.dma_start(xbig[:, kt, 512:1024], xT_d[ts(kt, 128), 512:1024])
            for kt in range(KT):
                nc.sync.dma_start(
                    xbig[:, kt, 1024:2048], xT_d[ts(kt, 128), 1024:2048]
                )
            for k2 in range(2):
                nc.sync.dma_start(wpt[:, k2, :], wp_d[ts(k2, 128), :])

            # PE prewarm: keep the tensor engine busy (and the HAM
            # clock-gate open) while the first input DMAs land.
            for w in range(32):
                psw = psj_pool.tile([128, 512], f32, tag="psj", name="psw")
                nc.tensor.matmul(
                    psw[:], zb[:, 0:128], zb[:, 0:512],
                    start=True, stop=True,
                )

            # ---------------- building blocks ----------------
            eTs = [dict() for _ in range(NH)]  # [h][kj] -> masked exp'd scores

            def proj_v(sb):
                vsb = vbig[:, sb, :].rearrange("p (h c) -> p h c", c=65)
                nc.vector.memset(vsb[:, :, 64:65], 1.0)
                psv = psj_pool.tile([128, 256], f32, tag="psj", name="psv")
                for kt in range(KT):
                    nc.tensor.matmul(
                        psv[:],
                        xbig[:, kt, ts(sb, 128)],
                        wbig[:, kt, ds(2 * NH * HD, NH * HD)],
                        start=(kt == 0),
                        stop=(kt == KT - 1),
                    )
                nc.vector.tensor_copy(
                    vsb[:, :, 0:64],
                    psv[:].rearrange("p (h c) -> p h c", c=64),
                )

            def proj_qk(c, sc):
                psb = psj_pool.tile([128, 512], f32, tag="psj", name="psb")
                for kt in range(KT):
                    nc.tensor.matmul(
                        psb[:],
                        wbig[:, kt, ts(c, 128)],
                        xbig[:, kt, ts(sc, 512)],
                        start=(kt == 0),
                        stop=(kt == KT - 1),
                    )
                nc.scalar.add(
                    qk[c][:, ts(sc, 512)], psb[:], bqk_t[:, c : c + 1]
                )
                # rope: rotate_half via partition-swap SBUF DMAs on the
                # sync/gpsimd queues (sign is folded into the sin table)
                qc = qk[c][:, ts(sc, 512)]
                tmp = tmp_pool.tile([128, 512], bf16, tag="ropetmp", name="tmp")
                if sc < 2:
                    # prologue: ACT is idle, lend its DMA queue to rope
                    dma_engs = [nc.sync, nc.gpsimd, nc.scalar, nc.gpsimd]
                else:
                    dma_engs = [nc.sync, nc.gpsimd, nc.sync, nc.gpsimd]
                for g in range(2):
                    b0 = g * 64
                    dma_engs[2 * g].dma_start(
                        tmp[b0 : b0 + 32, :],
                        qk[c][b0 + 32 : b0 + 64, ts(sc, 512)],
                    )
                    dma_engs[2 * g + 1].dma_start(
                        tmp[b0 + 32 : b0 + 64, :],
                        qk[c][b0 : b0 + 32, ts(sc, 512)],
                    )
                nc.vector.tensor_tensor(
                    tmp[:], tmp[:], sin2[:, ts(sc, 512)], op=mult
                )
                nc.vector.tensor_tensor(qc, qc, cos2[:, ts(sc, 512)], op=mult)
                nc.vector.tensor_add(qc, qc, tmp[:])
                if c >= 2:
                    # spread each roped k head into its padded K=128 tile
                    # (partition-aligned with the head's q rows)
                    h0 = 2 * (c - 2)
                    nc.sync.dma_start(
                        kp[h0][0:64, ts(sc, 512)], qk[c][0:64, ts(sc, 512)]
                    )
                    nc.gpsimd.dma_start(
                        kp[h0 + 1][64:128, ts(sc, 512)], qk[c][64:128, ts(sc, 512)]
                    )

            def scores_mm(h, kj, pss, nq, part):
                # one scores matmul sT[k, q]; part 0 = first 512 q-cols,
                # part 1 = the 128-col band tail. lhsT is the head's
                # zero-padded K=128 tile; the moving q carries both heads'
                # rows (the foreign head is nulled by the zero k rows).
                qt = qk[h // 2]
                n1 = min(512, nq * 128)
                n2 = nq * 128 - n1
                lhs_k = kp[h][:, ts(kj, 128)]
                if part == 0:
                    nc.tensor.matmul(
                        pss[:, 0:n1],
                        lhs_k,
                        qt[:, ds(kj * 128, n1)],
                        start=True,
                        stop=True,
                    )
                elif n2:
                    nc.tensor.matmul(
                        pss[:, 512 : 512 + n2],
                        lhs_k,
                        qt[:, ds(kj * 128 + 512, n2)],
                        start=True,
                        stop=True,
                    )

            def exp_mask(h, kj, pss, nq):
                eT = e_pool.tile([128, 640], bf16, tag="et", name="eT")
                nc.scalar.activation(
                    eT[:, 0 : nq * 128], pss[:, 0 : nq * 128], Exp, scale=0.125
                )
                # banded mask: diag block keeps kk<=qq, window edge kk>qq.
                # Both 128-col regions are masked in one strided DVE op.
                if nq == WB + 1:
                    e2 = eT[:, 0:640].rearrange("p (x y) -> p x y", y=128)[
                        :, 0:5:4, :
                    ]
                    nc.vector.tensor_tensor(e2, e2, m04t[:], op=mult)
                else:
                    nc.vector.tensor_tensor(
                        eT[:, 0:128], eT[:, 0:128], m04t[:, 0, :], op=mult
                    )
                eTs[h][kj] = eT

            def evac_q(po, h, qtr):
                # normalize by denominators (PSUM row 64) into outH.
                # The denom row is staged via ACT while ACT has slack
                # (early waves); via DVE once ACT is exp-bound late.
                hb = (h % 2) * 64
                rb = rb_pool.tile([64, HS], f32, tag="rb", name="rb")
                if qtr == 2:
                    nc.vector.tensor_copy(rb[0:1, :], po[64:65, :])
                else:
                    nc.scalar.copy(rb[0:1, :], po[64:65, :])
                nc.vector.reciprocal_approx_fast(rb[0:1, :], rb[0:1, :])
                nc.gpsimd.partition_broadcast(rb[:], rb[0:1, :])
                nc.vector.tensor_tensor(
                    outH[hb : hb + 64, h // 2, qtr * HS : (qtr + 1) * HS],
                    po[0:64, :],
                    rb[:],
                    op=mult,
                )

            def attnv(h, qtr):
                # kj-major attn@V: each V block loads once and streams its
                # whole q-span (per-element has_written bits handle the
                # staggered accumulation regions)
                kjlo = max(0, 4 * qtr - WB)
                kjhi = 4 * qtr + QB - 1
                po = pso_pool.tile([65, HS], f32, tag="pso", name="po")
                for kj in range(kjlo, kjhi + 1):
                    qlo = max(4 * qtr, kj)
                    qhi = min(4 * qtr + QB - 1, kj + WB)
                    off = (qlo - kj) * 128
                    n = (qhi - qlo + 1) * 128
                    nc.tensor.matmul(
                        po[:, ds((qlo - 4 * qtr) * 128, n)],
                        vbig[:, kj, h * 65 : h * 65 + 65],
                        eTs[h][kj][:, off : off + n],
                        start=(kj == kjlo),
                        stop=(kj == kjhi),
                    )
                evac_q(po, h, qtr)

            def cproj(sb):
                # two 1-bank psum groups so c_proj never contends with the
                # scores pool
                yt = y_pool.tile([128, D], bf16, tag="yo", name="yt")
                for ncol in range(2):
                    psp = psj_pool.tile([128, 512], f32, tag="psj", name="psp")
                    for k2 in range(2):
                        nc.tensor.matmul(
                            psp[:],
                            outH[:, k2, ts(sb, 128)],
                            wpt[:, k2, ts(ncol, 512)],
                            start=(k2 == 0),
                            stop=(k2 == 1),
                        )
                    if (sb + ncol) % 2 == 0:
                        nc.scalar.copy(yt[:, ts(ncol, 512)], psp[:])
                    else:
                        nc.vector.tensor_copy(yt[:, ts(ncol, 512)], psp[:])
                    # ship each half as soon as its evacuation lands
                    nc.sync.dma_start(
                        out_d[ts(sb, 128), ts(ncol, 512)], yt[:, ts(ncol, 512)]
                    )

            def scores_kj(kj):
                # scores+exp for one key block, head-pair interleaved
                nq = min(WB + 1, NB - kj)
                for pair in range(2):
                    h0, h1 = 2 * pair, 2 * pair + 1
                    p0 = ps_pool.tile([128, 640], f32, tag="ps", name="pss")
                    scores_mm(h0, kj, p0, nq, 0)
                    scores_mm(h0, kj, p0, nq, 1)
                    p1 = ps_pool.tile([128, 640], f32, tag="ps", name="pss")
                    scores_mm(h1, kj, p1, nq, 0)
                    scores_mm(h1, kj, p1, nq, 1)
                    exp_mask(h0, kj, p0, nq)
                    exp_mask(h1, kj, p1, nq)

            def wave(qtr):
                # Scores run a full wave ahead: this wave consumes quarter
                # qtr's eT tiles (exp'd during wave qtr-1) and produces
                # quarter qtr+1's. Projection chunks lead the wave so the
                # PE stays dense (and the clock-gate warm) while ACT drains
                # the exp backlog; attn@V heads, new score blocks, and the
                # previous quarter's c_proj interleave so no engine queue
                # builds a deep backlog in front of a dependency.
                ch = qtr + 2
                if ch < 4:
                    for j in range(4):
                        proj_v(4 * ch + j)
                        proj_qk((0, 2, 1, 3)[j], ch)
                # c_proj of the previous quarter starts one slot late so
                # its first group never waits on that quarter's last
                # normalization chain
                for i in range(4):
                    if qtr >= 1 and i >= 1:
                        cproj(4 * (qtr - 1) + i - 1)
                    attnv(i, qtr)
                    if qtr < 3:
                        scores_kj(4 * qtr + 4 + i)
                if qtr >= 1:
                    cproj(4 * (qtr - 1) + 3)

            # ---------------- pipelined schedule ----------------
            for sc in range(2):
                for sb in range(4 * sc, 4 * sc + 4):
                    proj_v(sb)
                for c in (0, 2, 1, 3):
                    proj_qk(c, sc)
            for kj in range(4):
                scores_kj(kj)
            for qtr in range(4):
                wave(qtr)
            for sb in range(12, 16):
                cproj(sb)

    nc.compile()
    return nc


def _host_inputs(hidden, pos, caw, cab, cpw):
    """Build the 8 per-core input maps."""
    inv = 1.0 / (ROPE_BASE ** (np.arange(0, HD, 2, dtype=np.float32) / HD))
    t = np.arange(S, dtype=np.float32)
    freqs = np.outer(t, inv).astype(np.float32)
    emb = np.concatenate([freqs, freqs], axis=1)  # [S, HD]
    cos = np.cos(emb).astype(np.float32)
    sin = np.sin(emb).astype(np.float32)

    import ml_dtypes

    bf = ml_dtypes.bfloat16
    ii = np.arange(128)
    m0 = (ii[:, None] <= ii[None, :]).astype(bf)
    m4 = (ii[:, None] > ii[None, :]).astype(bf)
    m04 = np.ascontiguousarray(np.concatenate([m0, m4], axis=1))

    xTs, cos2s, sin2s = [], [], []
    for b in range(B):
        xTs.append(np.ascontiguousarray(hidden[b].T).astype(bf))
        cosT = np.ascontiguousarray(cos[pos[b]].T)  # [HD, S]
        sinT = np.ascontiguousarray(sin[pos[b]].T)
        sinS = np.concatenate([-sinT[:32], sinT[32:]], axis=0)
        cos2s.append(np.tile(cosT, (2, 1)).astype(bf))
        sin2s.append(np.tile(sinS, (2, 1)).astype(bf))

    in_maps = []
    for c in range(NCORES):
        b = c // 4
        h0 = NH * (c % 4)
        col = h0 * HD
        w_q = caw[:, col : col + NH * HD]
        w_k = caw[:, D + col : D + col + NH * HD]
        w_v = caw[:, 2 * D + col : 2 * D + col + NH * HD]
        wqkv = np.ascontiguousarray(
            np.concatenate([w_q, w_k, w_v], axis=1)
        ).astype(bf)
        b_q = cab[col : col + NH * HD]
        b_k = cab[D + col : D + col + NH * HD]
        bqk = np.ascontiguousarray(
            np.concatenate([b_q, b_k]).reshape(4, 128).T
        )  # [128, 4]: partition = col within tile
        wp = np.ascontiguousarray(cpw[col : col + NH * HD, :]).astype(bf)
        in_maps.append(
            {
                "xT": xTs[b],
                "wqkv": wqkv,
                "bqk": bqk,
                "wp": wp,
                "cos2": cos2s[b],
                "sin2": sin2s[b],
                "m04": m04,
            }
        )
    return in_maps


def _assemble(results, cab, cpw, cpb):
    """Host all-reduce of the 4 per-batch partials + biases.

    The V-bias contribution is position-independent after softmax
    (attn rows sum to 1), so it folds into a constant row vector:
    bias_v @ c_proj_w.
    """
    vrow = cab[2 * D :].astype(np.float32) @ cpw.astype(np.float32)
    bias = cpb.astype(np.float32) + vrow
    y = np.empty((B, S, D), dtype=np.float32)
    for b in range(B):
        acc = results[4 * b]["out"].astype(np.float32)
        for c in range(4 * b + 1, 4 * b + 4):
            acc = acc + results[c]["out"].astype(np.float32)
        y[b] = acc + bias[None, :]
    return y


def kernel(**inputs):
    from concourse import bass_utils

    hidden = np.asarray(inputs["hidden_states"], dtype=np.float32)
    pos = np.asarray(inputs["position_ids"]).astype(np.int64)
    caw = np.asarray(inputs["c_attn_w"], dtype=np.float32)
    cab = np.asarray(inputs["c_attn_b"], dtype=np.float32)
    cpw = np.asarray(inputs["c_proj_w"], dtype=np.float32)
    cpb = np.asarray(inputs["c_proj_b"], dtype=np.float32)

    in_maps = _host_inputs(hidden, pos, caw, cab, cpw)
    nc = _build_nc()
    res = bass_utils.run_bass_kernel_spmd(nc, in_maps, list(range(NCORES)))
    return _assemble(res.results, cab, cpw, cpb)
